# revision 6
# baseline (speedup 1.0000x reference)
"""BitConv2d forward on 8 Trainium2 NeuronCores (SPMD data-parallel).

Strategy:
  - Shard batch (32) -> 4 images per core; replicate the tiny bit-plane
    weights/scales on every core. No collectives needed (forward only).
  - Host precomputes the integer conv weights
        W_int[o,i,kh,kw] = sum_b (pweight-nweight)[...,b] * 2^(3-b)   (exact, in [-15,15])
    and ships them as per-tap block-diagonal stationary operands
    [[W_t,0],[0,W_t]]^T (128x128, bf16 - exact for ints <= 15), plus the
    fused scale (scale/15) and bias vectors. No on-device weight prep.
  - Host pre-pads each image into the exact SBUF layout the matmuls read
    ([128, 59*114]: partitions 0:64 = padded rows 0..57, partitions
    64:128 = padded rows 55..112+zero, row-flattened at stride 114, zero
    pad columns included), in bf16. Optionally also an fp8(e4m3) copy
    [128, 2, XCA] holding (x, x-shifted-by-1) for DoubleRow tap pairs.
  - 3x3 same-pad conv as accumulating matmuls per 512-col output tile:
    either 9 bf16 taps, or 5 bf16 taps + 2 fp8 DoubleRow matmuls that
    each fold a pair of horizontally-adjacent taps (contraction 256).
  - Epilogue on ACT: out = psum*(scale/15) + bias, cast to fp16; stores
    stream out per tile. All DMAs are issued in ~1KB-per-partition
    column chunks so descriptors rotate across partitions (per-partition
    SBUF port serializes big descriptors; tiny ones are rate-bound).
  - Host reassembles the raw [128, 56*114] fp16 tiles to NCHW f32.
"""

import numpy as np
import ml_dtypes

B, C, H, W = 32, 64, 112, 112
NB = 4
CORES = 8
BPC = B // CORES  # images per core

WP = H + 2  # padded width/height = 114
HALF = H // 2  # 56 output rows per position-group
XC_DATA = 58 * WP  # 6612 data columns per partition block
XC = 59 * WP  # + one zero row (junk-column tap reads run past the data)
XCA = 6736  # fp8 copy stride, 16B-aligned
OUTC = HALF * WP  # 6384 output columns per group

# N-tiles: all >=256 for full PE rate (PSUM bank caps at 512 fp32)
N_TILES = [(i * 512, 512) for i in range(11)] + [(5632, 376), (6008, 376)]
TAP_OFFS = [kh * WP + kw for kh in range(3) for kw in range(3)]

USE_FP8 = True
FP8_PAIRS = [(0, 1), (3, 4)]  # horizontally adjacent taps, offset delta = 1
FP8_TAPS = [t for p in FP8_PAIRS for t in p]
BF16_TAPS = [t for t in range(9) if t not in FP8_TAPS]

LOAD_CHUNK = 512  # elements per DMA chunk (~1KB/partition descriptors)

_CACHE = {}


def _build():
    key = ("nc", USE_FP8)
    if key in _CACHE:
        return _CACHE[key]
    import concourse.bacc as bacc
    import concourse.mybir as mybir
    from concourse import tile

    f32 = mybir.dt.float32
    f16 = mybir.dt.float16
    bf16 = mybir.dt.bfloat16
    f8 = mybir.dt.float8e4

    nc = bacc.Bacc("TRN2", target_bir_lowering=False, debug=False, num_devices=CORES)

    xp_d = nc.dram_tensor("xp", [BPC, 128, XC], bf16, kind="ExternalInput").ap()
    if USE_FP8:
        xq_d = nc.dram_tensor("xq", [BPC, 128, 2, XCA], f8, kind="ExternalInput").ap()
        wl8_d = nc.dram_tensor("wl8", [len(FP8_PAIRS), 128, 2, 128], f8, kind="ExternalInput").ap()
    wl_d = nc.dram_tensor("wl", [128, 9 * 128], bf16, kind="ExternalInput").ap()
    sc_d = nc.dram_tensor("scalev", [128, 1], f32, kind="ExternalInput").ap()
    bi_d = nc.dram_tensor("biasv", [128, 1], f32, kind="ExternalInput").ap()
    y_d = nc.dram_tensor("y", [BPC, 128, OUTC], f16, kind="ExternalOutput").ap()

    with tile.TileContext(nc) as tc:
        with (
            tc.tile_pool(name="consts", bufs=1) as consts,
            tc.tile_pool(name="xpool", bufs=3) as xpool,
            tc.tile_pool(name="opool", bufs=2) as opool,
            tc.tile_pool(name="pspool", bufs=8, space="PSUM") as pspool,
        ):
            # ---- consts: stationary weights + epilogue vectors ----
            wl = consts.tile([128, 9 * 128], bf16, tag="wl")
            for c0 in range(0, 9 * 128, 512):
                c1 = min(c0 + 512, 9 * 128)
                nc.sync.dma_start(wl[:, c0:c1], wl_d[:, c0:c1])
            lhsT = [wl[:, t * 128 : (t + 1) * 128] for t in range(9)]
            if USE_FP8:
                wl8 = consts.tile([128, len(FP8_PAIRS), 2, 128], f8, tag="wl8")
                for p in range(len(FP8_PAIRS)):
                    nc.sync.dma_start(wl8[:, p], wl8_d[p])
            scale_vec = consts.tile([128, 1], f32, tag="scale_vec")
            bias_vec = consts.tile([128, 1], f32, tag="bias_vec")
            nc.sync.dma_start(scale_vec[:], sc_d)
            nc.sync.dma_start(bias_vec[:], bi_d)

            # ---- image load pipeline (host-prepadded, chunked DMAs) ----
            def load_image(b):
                xs = xpool.tile([128, XC], bf16, tag="xs", name=f"xs{b}")
                for c0 in range(0, XC, LOAD_CHUNK):
                    c1 = min(c0 + LOAD_CHUNK, XC)
                    nc.gpsimd.dma_start(xs[:, c0:c1], xp_d[b, :, c0:c1])
                if not USE_FP8:
                    return xs, None
                xq = xpool.tile([128, 2, XCA], f8, tag="xq", name=f"xq{b}")
                for j in range(2):
                    for c0 in range(0, XCA, 2 * LOAD_CHUNK):
                        c1 = min(c0 + 2 * LOAD_CHUNK, XCA)
                        nc.gpsimd.dma_start(xq[:, j, c0:c1], xq_d[b, :, j, c0:c1])
                return xs, xq

            img_next = load_image(0)
            img_next2 = load_image(1)

            # ---- main conv loop ----
            for b in range(BPC):
                xs, xq = img_next
                img_next = img_next2
                img_next2 = load_image(b + 2) if b + 2 < BPC else None

                outb = opool.tile([128, OUTC], f16, tag="outb")
                for n0, nt in N_TILES:
                    ps = pspool.tile([128, 512], f32, tag="ps")
                    if USE_FP8:
                        for pi, (t0, _t1) in enumerate(FP8_PAIRS):
                            nc.tensor.matmul(
                                ps[:, 0:nt],
                                wl8[:, pi],
                                xq[:, :, n0 + TAP_OFFS[t0] : n0 + TAP_OFFS[t0] + nt],
                                start=(pi == 0),
                                stop=False,
                                perf_mode=mybir.MatmulPerfMode.DoubleRow,
                            )
                        taps = BF16_TAPS
                    else:
                        taps = range(9)
                    for i, t in enumerate(taps):
                        off = TAP_OFFS[t]
                        nc.tensor.matmul(
                            ps[:, 0:nt],
                            lhsT[t],
                            xs[:, n0 + off : n0 + off + nt],
                            start=(not USE_FP8 and i == 0),
                            stop=(i == len(taps) - 1),
                        )
                    nc.scalar.activation(
                        outb[:, n0 : n0 + nt],
                        ps[:, 0:nt],
                        mybir.ActivationFunctionType.Identity,
                        bias=bias_vec[:],
                        scale=scale_vec[:],
                    )
                    nc.sync.dma_start(y_d[b, :, n0 : n0 + nt], outb[:, n0 : n0 + nt])

    nc.compile()
    _CACHE[key] = nc
    return nc


def _pad_layout(x):
    """[B, 64, 112, 112] f32 -> [B, 128, XC] f32 padded SBUF layout."""
    xp = np.zeros((x.shape[0], 128, XC), dtype=np.float32)
    v = xp[:, :, :XC_DATA].reshape(x.shape[0], 128, 58, WP)
    # block 0: padded rows 0..57 hold image rows -1..56 (row r = image row r-1)
    v[:, 0:C, 1:58, 1 : 1 + W] = x[:, :, 0:57, :]
    # block 1: rows 0..56 hold image rows 55..111, row 57 stays zero
    v[:, C:128, 0:57, 1 : 1 + W] = x[:, :, 55:112, :]
    return xp


def _prep_inputs(inputs):
    x = np.asarray(inputs["x"], dtype=np.float32)
    pw = np.asarray(inputs["pweight"], np.float32)
    nw = np.asarray(inputs["nweight"], np.float32)
    pb = np.asarray(inputs["pbias"], np.float32)
    nb = np.asarray(inputs["nbias"], np.float32)
    scale = np.asarray(inputs["scale"], np.float32)[0]
    bscale = np.asarray(inputs["biasscale"], np.float32)[0]

    exps2 = np.array([8.0, 4.0, 2.0, 1.0], np.float32)
    wint = ((pw - nw) * exps2).sum(-1)  # [O, I, 3, 3], exact ints in [-15, 15]
    bias = ((pb - nb) * exps2).sum(-1) * (bscale / 15.0)  # [O]

    # per-tap block-diagonal transposed stationary operands
    wl = np.zeros((128, 9, 128), np.float32)
    for t in range(9):
        kh, kw = divmod(t, 3)
        wt = wint[:, :, kh, kw].T  # [I, O] = lhsT block
        wl[0:C, t, 0:C] = wt
        wl[C:128, t, C:128] = wt
    wl = wl.reshape(128, 9 * 128).astype(ml_dtypes.bfloat16)

    scale_vec = np.full((128, 1), scale / 15.0, np.float32)
    bias_vec = np.concatenate([bias, bias]).reshape(128, 1).astype(np.float32)

    xpad = _pad_layout(x)
    shared = {
        "wl": wl,
        "scalev": scale_vec,
        "biasv": bias_vec,
    }
    if USE_FP8:
        e4 = ml_dtypes.float8_e4m3
        xq = np.zeros((B, 128, 2, XCA), e4)
        xq[:, :, 0, :XC] = xpad.astype(e4)
        xq[:, :, 1, : XC - 1] = xpad[:, :, 1:].astype(e4)
        wl8 = np.zeros((len(FP8_PAIRS), 128, 2, 128), e4)
        for p, (t0, t1) in enumerate(FP8_PAIRS):
            for j, t in enumerate((t0, t1)):
                kh, kw = divmod(t, 3)
                wt = wint[:, :, kh, kw].T
                wl8[p, 0:C, j, 0:C] = wt.astype(e4)
                wl8[p, C:128, j, C:128] = wt.astype(e4)
        shared["wl8"] = wl8

    xpad16 = xpad.astype(ml_dtypes.bfloat16)
    maps = []
    for c in range(CORES):
        m = dict(shared, xp=np.ascontiguousarray(xpad16[c * BPC : (c + 1) * BPC]))
        if USE_FP8:
            m["xq"] = np.ascontiguousarray(xq[c * BPC : (c + 1) * BPC])
        maps.append(m)
    return maps


def _assemble(results):
    """Raw [BPC, 128, 6384] fp16 per core -> [B, 64, 112, 112] f32."""
    out = np.empty((B, C, H, W), dtype=np.float32)
    for c in range(CORES):
        raw = np.asarray(results[c]["y"], dtype=np.float32).reshape(
            BPC, 128, HALF, WP
        )
        out[c * BPC : (c + 1) * BPC, :, 0:HALF, :] = raw[:, 0:C, :, 0:W]
        out[c * BPC : (c + 1) * BPC, :, HALF:H, :] = raw[:, C:128, :, 0:W]
    return out


def _run(inputs, trace=False):
    from concourse.bass_utils import run_bass_kernel_spmd

    nc = _build()
    in_maps = _prep_inputs(inputs)
    last_err = None
    for attempt in range(3):
        try:
            res = run_bass_kernel_spmd(
                nc, in_maps, core_ids=list(range(CORES)), trace=trace
            )
            return _assemble(res.results), res.exec_time_ns
        except Exception as e:  # transient NRT_EXEC_UNIT_UNRECOVERABLE recovers on retry
            last_err = e
            import time

            time.sleep(10)
    raise last_err


def kernel(**inputs) -> np.ndarray:
    out, _ = _run(inputs)
    return out


# revision 14
# speedup vs baseline: 1.0861x; 1.0861x over previous
"""BitConv2d forward on 8 Trainium2 NeuronCores (SPMD data-parallel).

Strategy:
  - Shard batch (32) -> 4 images per core; replicate the tiny bit-plane
    weights/scales on every core. No collectives needed (forward only).
  - Host precomputes the integer conv weights
        W_int[o,i,kh,kw] = sum_b (pweight-nweight)[...,b] * 2^(3-b)   (exact, in [-15,15])
    and ships them as per-tap block-diagonal stationary operands
    [[W_t,0],[0,W_t]]^T (128x128, bf16 - exact for ints <= 15), plus the
    fused scale (scale/15) and bias vectors. No on-device weight prep.
  - Host pre-pads each image into the exact SBUF layout the matmuls read
    ([128, 59*114]: partitions 0:64 = padded rows 0..57, partitions
    64:128 = padded rows 55..112+zero, row-flattened at stride 114, zero
    pad columns included), in bf16. Optionally also an fp8(e4m3) copy
    [128, 2, XCA] holding (x, x-shifted-by-1) for DoubleRow tap pairs.
  - 3x3 same-pad conv as accumulating matmuls per 512-col output tile:
    either 9 bf16 taps, or 5 bf16 taps + 2 fp8 DoubleRow matmuls that
    each fold a pair of horizontally-adjacent taps (contraction 256).
  - Epilogue on ACT: out = psum*(scale/15) + bias, cast to fp16; stores
    stream out per tile. All DMAs are issued in ~1KB-per-partition
    column chunks so descriptors rotate across partitions (per-partition
    SBUF port serializes big descriptors; tiny ones are rate-bound).
  - Host reassembles the raw [128, 56*114] fp16 tiles to NCHW f32.
"""

import numpy as np
import ml_dtypes

B, C, H, W = 32, 64, 112, 112
NB = 4
CORES = 8
BPC = B // CORES  # images per core

WP = H + 2  # padded width/height = 114
HALF = H // 2  # 56 output rows per position-group
XC_DATA = 58 * WP  # 6612 data columns per partition block
XC = 59 * WP  # + one zero row (junk-column tap reads run past the data)
XCA = 6736  # fp8 copy stride, 16B-aligned
OUTC = HALF * WP  # 6384 output columns per group

# N-tiles: all >=256 for full PE rate (PSUM bank caps at 512 fp32)
N_TILES = [(i * 512, 512) for i in range(11)] + [(5632, 376), (6008, 376)]
TAP_OFFS = [kh * WP + kw for kh in range(3) for kw in range(3)]

USE_FP8 = False
EXPT_SPLIT = True  # split block-diag MMs into 2 concurrent col-group MMs
FP8_PAIRS = [(0, 1), (3, 4)]  # horizontally adjacent taps, offset delta = 1
FP8_TAPS = [t for p in FP8_PAIRS for t in p]
BF16_TAPS = [t for t in range(9) if t not in FP8_TAPS]

LOAD_CHUNK = 512  # elements per DMA chunk (~1KB/partition descriptors)

_CACHE = {}


def _build():
    key = ("nc", USE_FP8, EXPT_SPLIT)
    if key in _CACHE:
        return _CACHE[key]
    import concourse.bacc as bacc
    import concourse.mybir as mybir
    from concourse import tile

    f32 = mybir.dt.float32
    f16 = mybir.dt.float16
    bf16 = mybir.dt.bfloat16
    f8 = mybir.dt.float8e4

    nc = bacc.Bacc("TRN2", target_bir_lowering=False, debug=False, num_devices=CORES)

    xp_d = nc.dram_tensor("xp", [BPC, 128, XC], bf16, kind="ExternalInput").ap()
    if USE_FP8:
        xq_d = nc.dram_tensor("xq", [BPC, 128, 2, XCA], f8, kind="ExternalInput").ap()
        wl8_d = nc.dram_tensor("wl8", [len(FP8_PAIRS), 128, 2, 128], f8, kind="ExternalInput").ap()
    wl_d = nc.dram_tensor("wl", [128, 9 * 128], bf16, kind="ExternalInput").ap()
    if EXPT_SPLIT:
        ws_d = nc.dram_tensor("ws", [128, 9 * 64], bf16, kind="ExternalInput").ap()
    sc_d = nc.dram_tensor("scalev", [128, 1], f32, kind="ExternalInput").ap()
    bi_d = nc.dram_tensor("biasv", [128, 1], f32, kind="ExternalInput").ap()
    y_d = nc.dram_tensor("y", [BPC, 128, OUTC], f16, kind="ExternalOutput").ap()

    with tile.TileContext(nc) as tc:
        with (
            tc.tile_pool(name="consts", bufs=1) as consts,
            tc.tile_pool(name="xpool", bufs=3) as xpool,
            tc.tile_pool(name="opool", bufs=2) as opool,
            tc.tile_pool(name="pspool", bufs=8, space="PSUM") as pspool,
        ):
            # ---- consts: stationary weights + epilogue vectors ----
            wl = consts.tile([128, 9 * 128], bf16, tag="wl")
            for c0 in range(0, 9 * 128, 512):
                c1 = min(c0 + 512, 9 * 128)
                nc.sync.dma_start(wl[:, c0:c1], wl_d[:, c0:c1])
            lhsT = [wl[:, t * 128 : (t + 1) * 128] for t in range(9)]
            if EXPT_SPLIT:
                ws = consts.tile([128, 9 * 64], bf16, tag="ws")
                nc.sync.dma_start(ws[:, 0:512], ws_d[:, 0:512])
                nc.sync.dma_start(ws[:, 512:576], ws_d[:, 512:576])
            if USE_FP8:
                wl8 = consts.tile([128, len(FP8_PAIRS), 2, 128], f8, tag="wl8")
                for p in range(len(FP8_PAIRS)):
                    nc.sync.dma_start(wl8[:, p], wl8_d[p])
            scale_vec = consts.tile([128, 1], f32, tag="scale_vec")
            bias_vec = consts.tile([128, 1], f32, tag="bias_vec")
            nc.sync.dma_start(scale_vec[:], sc_d)
            nc.sync.dma_start(bias_vec[:], bi_d)

            # ---- image load pipeline (host-prepadded, chunked DMAs) ----
            def load_image(b):
                xs = xpool.tile([128, XC], bf16, tag="xs", name=f"xs{b}")
                for c0 in range(0, XC, LOAD_CHUNK):
                    c1 = min(c0 + LOAD_CHUNK, XC)
                    nc.gpsimd.dma_start(xs[:, c0:c1], xp_d[b, :, c0:c1])
                if not USE_FP8:
                    return xs, None
                xq = xpool.tile([128, 2, XCA], f8, tag="xq", name=f"xq{b}")
                for j in range(2):
                    for c0 in range(0, XCA, 2 * LOAD_CHUNK):
                        c1 = min(c0 + 2 * LOAD_CHUNK, XCA)
                        nc.gpsimd.dma_start(xq[:, j, c0:c1], xq_d[b, :, j, c0:c1])
                return xs, xq

            img_next = load_image(0)
            img_next2 = load_image(1)

            # ---- main conv loop ----
            for b in range(BPC):
                xs, xq = img_next
                img_next = img_next2
                img_next2 = load_image(b + 2) if b + 2 < BPC else None

                outb = opool.tile([128, OUTC], f16, tag="outb")
                for n0, nt in N_TILES:
                    ps = pspool.tile([128, 512], f32, tag="ps")
                    if USE_FP8:
                        for pi, (t0, _t1) in enumerate(FP8_PAIRS):
                            nc.tensor.matmul(
                                ps[:, 0:nt],
                                wl8[:, pi],
                                xq[:, :, n0 + TAP_OFFS[t0] : n0 + TAP_OFFS[t0] + nt],
                                start=(pi == 0),
                                stop=False,
                                perf_mode=mybir.MatmulPerfMode.DoubleRow,
                            )
                        taps = BF16_TAPS
                    else:
                        taps = range(9)
                    for i, t in enumerate(taps):
                        off = TAP_OFFS[t]
                        if EXPT_SPLIT:
                            nc.tensor.matmul(
                                ps[0:C, 0:nt],
                                ws[0:C, t * C : (t + 1) * C],
                                xs[0:C, n0 + off : n0 + off + nt],
                                start=(i == 0),
                                stop=(i == len(taps) - 1),
                                tile_position=(0, 0),
                            )
                            nc.tensor.matmul(
                                ps[C:128, 0:nt],
                                ws[C:128, t * C : (t + 1) * C],
                                xs[C:128, n0 + off : n0 + off + nt],
                                start=(i == 0),
                                stop=(i == len(taps) - 1),
                                tile_position=(64, 64),
                            )
                        else:
                            nc.tensor.matmul(
                                ps[:, 0:nt],
                                lhsT[t],
                                xs[:, n0 + off : n0 + off + nt],
                                start=(not USE_FP8 and i == 0),
                                stop=(i == len(taps) - 1),
                            )
                    nc.scalar.activation(
                        outb[:, n0 : n0 + nt],
                        ps[:, 0:nt],
                        mybir.ActivationFunctionType.Identity,
                        bias=bias_vec[:],
                        scale=scale_vec[:],
                    )
                    nc.sync.dma_start(y_d[b, :, n0 : n0 + nt], outb[:, n0 : n0 + nt])

    nc.compile()
    _CACHE[key] = nc
    return nc


def _pad_layout(x):
    """[B, 64, 112, 112] f32 -> [B, 128, XC] f32 padded SBUF layout."""
    xp = np.zeros((x.shape[0], 128, XC), dtype=np.float32)
    v = xp[:, :, :XC_DATA].reshape(x.shape[0], 128, 58, WP)
    # block 0: padded rows 0..57 hold image rows -1..56 (row r = image row r-1)
    v[:, 0:C, 1:58, 1 : 1 + W] = x[:, :, 0:57, :]
    # block 1: rows 0..56 hold image rows 55..111, row 57 stays zero
    v[:, C:128, 0:57, 1 : 1 + W] = x[:, :, 55:112, :]
    return xp


def _prep_inputs(inputs):
    x = np.asarray(inputs["x"], dtype=np.float32)
    pw = np.asarray(inputs["pweight"], np.float32)
    nw = np.asarray(inputs["nweight"], np.float32)
    pb = np.asarray(inputs["pbias"], np.float32)
    nb = np.asarray(inputs["nbias"], np.float32)
    scale = np.asarray(inputs["scale"], np.float32)[0]
    bscale = np.asarray(inputs["biasscale"], np.float32)[0]

    exps2 = np.array([8.0, 4.0, 2.0, 1.0], np.float32)
    wint = ((pw - nw) * exps2).sum(-1)  # [O, I, 3, 3], exact ints in [-15, 15]
    bias = ((pb - nb) * exps2).sum(-1) * (bscale / 15.0)  # [O]

    # per-tap block-diagonal transposed stationary operands
    wl = np.zeros((128, 9, 128), np.float32)
    for t in range(9):
        kh, kw = divmod(t, 3)
        wt = wint[:, :, kh, kw].T  # [I, O] = lhsT block
        wl[0:C, t, 0:C] = wt
        wl[C:128, t, C:128] = wt
    wl = wl.reshape(128, 9 * 128).astype(ml_dtypes.bfloat16)

    ws = np.zeros((128, 9, C), np.float32)
    for t in range(9):
        kh, kw = divmod(t, 3)
        wt = wint[:, :, kh, kw].T  # [I, O]
        ws[0:C, t] = wt
        ws[C:128, t] = wt
    ws = ws.reshape(128, 9 * C).astype(ml_dtypes.bfloat16)

    scale_vec = np.full((128, 1), scale / 15.0, np.float32)
    bias_vec = np.concatenate([bias, bias]).reshape(128, 1).astype(np.float32)

    xpad = _pad_layout(x)
    shared = {
        "wl": wl,
        "scalev": scale_vec,
        "biasv": bias_vec,
    }
    if EXPT_SPLIT:
        shared["ws"] = ws
    if USE_FP8:
        e4 = ml_dtypes.float8_e4m3
        xq = np.zeros((B, 128, 2, XCA), e4)
        xq[:, :, 0, :XC] = xpad.astype(e4)
        xq[:, :, 1, : XC - 1] = xpad[:, :, 1:].astype(e4)
        wl8 = np.zeros((len(FP8_PAIRS), 128, 2, 128), e4)
        for p, (t0, t1) in enumerate(FP8_PAIRS):
            for j, t in enumerate((t0, t1)):
                kh, kw = divmod(t, 3)
                wt = wint[:, :, kh, kw].T
                wl8[p, 0:C, j, 0:C] = wt.astype(e4)
                wl8[p, C:128, j, C:128] = wt.astype(e4)
        shared["wl8"] = wl8

    xpad16 = xpad.astype(ml_dtypes.bfloat16)
    maps = []
    for c in range(CORES):
        m = dict(shared, xp=np.ascontiguousarray(xpad16[c * BPC : (c + 1) * BPC]))
        if USE_FP8:
            m["xq"] = np.ascontiguousarray(xq[c * BPC : (c + 1) * BPC])
        maps.append(m)
    return maps


def _assemble(results):
    """Raw [BPC, 128, 6384] fp16 per core -> [B, 64, 112, 112] f32."""
    out = np.empty((B, C, H, W), dtype=np.float32)
    for c in range(CORES):
        raw = np.asarray(results[c]["y"], dtype=np.float32).reshape(
            BPC, 128, HALF, WP
        )
        out[c * BPC : (c + 1) * BPC, :, 0:HALF, :] = raw[:, 0:C, :, 0:W]
        out[c * BPC : (c + 1) * BPC, :, HALF:H, :] = raw[:, C:128, :, 0:W]
    return out


def _run(inputs, trace=False):
    from concourse.bass_utils import run_bass_kernel_spmd

    nc = _build()
    in_maps = _prep_inputs(inputs)
    last_err = None
    for attempt in range(3):
        try:
            res = run_bass_kernel_spmd(
                nc, in_maps, core_ids=list(range(CORES)), trace=trace
            )
            return _assemble(res.results), res.exec_time_ns
        except Exception as e:  # transient NRT_EXEC_UNIT_UNRECOVERABLE recovers on retry
            last_err = e
            import time

            time.sleep(10)
    raise last_err


def kernel(**inputs) -> np.ndarray:
    out, _ = _run(inputs)
    return out
